# revision 1
# baseline (speedup 1.0000x reference)
"""Trainium2 Bass kernel for nn_GATSTEMEncoder (2-layer GAT + Linear 1024->25088).

Self-contained: hardcodes all shapes; builds + compiles the Bass program on
first call (cached per graph structure) and runs it SPMD on 8 NeuronCores.

Sharding: nodes relabeled so core c owns new ids [c*1280,(c+1)*1280), degree-
sorted within core. Edges live with their dst core as a slot-CSR (slot 0 =
self loop). Per-edge gathers use dma_gather on an AllGathered feature table
T [10241, 1088] = [xw(1024) | al_src(4) | al_dst(4) | pad]; row 10240 is a pad
row with al_src = -1e9 so padded slots contribute exp()==0 exactly.
Final Linear is row-sharded: each core computes its 1280 rows x 25088 cols.
"""
import os
import sys
import numpy as np
import ml_dtypes

for p in ("/opt/trn_rl_repo", "/root/.axon_site", "/root/.axon_site/_ro/trn_rl_repo"):
    if p not in sys.path:
        sys.path.append(p)

H, C = 4, 256
HC = H * C
N = 10000
NPAD = 10240
NCORES = 8
SHARD = NPAD // NCORES          # 1280
NT = SHARD // 128               # 10 tiles/core
SHARD1 = 1281                   # shard rows incl. inline pad row
TROWS = NCORES * SHARD1         # 10248 table rows
PAD_ROW = 1280                  # pad row of shard 0 (any shard works)
NEG = -1.0e9
D_IN = 128
E_DIM = 16
FCOLS = 1088
FOUT = 25088
G = 8                           # slot-chunks per dma_gather call
NCH = 512                       # final matmul N-chunk (25088 = 49*512)
NNCH = FOUT // NCH


# ----------------------------------------------------------------- host prep

def _fold_weights(W, a_src, a_dst):
    din = W.shape[0]
    Wr = W.reshape(din, H, C)
    W_ext = np.zeros((din, FCOLS), np.float32)
    W_ext[:, :HC] = W
    W_ext[:, HC:HC + H] = np.einsum('dhc,hc->dh', Wr, a_src)
    W_ext[:, HC + H:HC + 2 * H] = np.einsum('dhc,hc->dh', Wr, a_dst)
    return W_ext


def _fold_edge(We, a_edge):
    return np.einsum('dhc,hc->dh', We.reshape(E_DIM, H, C), a_edge).astype(np.float32)


def _build_shards(edge_index, edge_features):
    src = np.asarray(edge_index[0], np.int64)
    dst = np.asarray(edge_index[1], np.int64)

    order = np.argsort(dst, kind='stable')
    src_s = src[order]
    counts = np.bincount(dst[order], minlength=N)
    starts = np.concatenate([[0], np.cumsum(counts)])
    counts_pad = np.concatenate([counts, np.zeros(NPAD - N, np.int64)])

    perm = np.empty(NPAD, np.int64)
    for c in range(NCORES):
        lo = c * SHARD
        d = counts_pad[lo:lo + SHARD]
        perm[lo:lo + SHARD] = lo + np.argsort(-d, kind='stable')
    inv = np.empty(NPAD, np.int64)
    inv[perm] = np.arange(NPAD)
    deg_new = counts_pad[perm]

    KT = np.zeros(NT, np.int64)
    for t in range(NT):
        mx = 0
        for c in range(NCORES):
            d = deg_new[c * SHARD + t * 128: c * SHARD + (t + 1) * 128]
            mx = max(mx, int(d.max()))
        KT[t] = mx + 1
    KT[-1] += (-int(KT.sum())) % G
    S = int(KT.sum()) * 128

    def remap(i):
        return (i // SHARD) * SHARD1 + (i % SHARD)

    shards = []
    for c in range(NCORES):
        src_slots = np.full(S, PAD_ROW, np.int64)
        ea_slots = np.zeros((S, E_DIM), np.float32)
        base = 0
        for t in range(NT):
            kt = int(KT[t])
            for p in range(128):
                nid_new = c * SHARD + t * 128 + p
                nid_old = perm[nid_new]
                if nid_old >= N:
                    continue
                src_slots[base + p] = remap(nid_new)
                d = int(counts_pad[nid_old])
                if d > 0:
                    e0 = starts[nid_old]
                    idxs = base + (np.arange(d) + 1) * 128 + p
                    src_slots[idxs] = remap(inv[src_s[e0:e0 + d]])
                    ea_slots[idxs] = edge_features[order[e0:e0 + d]]
            base += kt * 128
        shards.append((src_slots.astype(np.int32), ea_slots,
                       deg_new[c * SHARD:(c + 1) * SHARD].astype(np.float32)))
    return shards, KT, S, perm, inv


# --------------------------------------------------------------- bass build

_CACHE = {}


def _build(KT, S, stop_after=99):
    import concourse.bass as bass
    import concourse.mybir as mybir
    import concourse.tile as tile
    from concourse import bacc
    from concourse.masks import make_identity

    f32 = mybir.dt.float32
    bf16 = mybir.dt.bfloat16
    i16 = mybir.dt.int16
    Ctot = S // 128
    KTmax = int(max(KT))
    rg = [list(range(NCORES))]

    nc = bacc.Bacc("TRN2", target_bir_lowering=False, debug=False,
                   num_devices=NCORES)

    # -------- I/O
    xT_d = nc.dram_tensor("x_T", [D_IN, SHARD], f32, kind="ExternalInput")
    W1e_d = nc.dram_tensor("W1e", [D_IN, FCOLS], f32, kind="ExternalInput")
    W2e_d = nc.dram_tensor("W2e", [HC, FCOLS], bf16, kind="ExternalInput")
    Mbc_d = nc.dram_tensor("M_bc", [128, 2, H, E_DIM], f32, kind="ExternalInput")
    bbc_d = nc.dram_tensor("b_bc", [128, 2, HC], f32, kind="ExternalInput")
    deg_d = nc.dram_tensor("deg", [128, NT], f32, kind="ExternalInput")
    idx_d = nc.dram_tensor("idx16", [128, S // 16], i16, kind="ExternalInput")
    ea_d = nc.dram_tensor("ea_sl", [S, E_DIM], f32, kind="ExternalInput")
    Wf_d = nc.dram_tensor("Wf", [HC, FOUT], bf16, kind="ExternalInput")
    bf_d = nc.dram_tensor("bf_bc", [128, FOUT], f32, kind="ExternalInput")
    out_d = nc.dram_tensor("out", [SHARD, FOUT], f32, kind="ExternalOutput")

    with tile.TileContext(nc) as tc:
        with (
            tc.tile_pool(name="const", bufs=1) as cpool,
            tc.tile_pool(name="dram", bufs=1, space="DRAM") as dpool,
            tc.tile_pool(name="persist", bufs=1) as ppool,
        ):
            # ---- constants
            ident = cpool.tile([128, 128], f32, tag="ident")
            make_identity(nc, ident[:])
            identb = cpool.tile([128, 128], bf16, tag="identb")
            make_identity(nc, identb[:])
            ones_row = cpool.tile([1, 128], f32, tag="ones_row")
            nc.vector.memset(ones_row[:], 1.0)
            padrow = cpool.tile([1, FCOLS], f32, tag="padrow")
            nc.vector.memset(padrow[:], 0.0)
            nc.vector.memset(padrow[:, HC:HC + H], NEG)
            Mbc = cpool.tile([128, 2, H, E_DIM], f32, tag="Mbc")
            nc.sync.dma_start(out=Mbc[:], in_=Mbc_d.ap())
            bbc = cpool.tile([128, 2, HC], f32, tag="bbc")
            nc.sync.dma_start(out=bbc[:], in_=bbc_d.ap())
            deg_sb = cpool.tile([128, NT], f32, tag="deg")
            nc.sync.dma_start(out=deg_sb[:], in_=deg_d.ap())
            deg_inv = cpool.tile([128, NT], f32, tag="deg_inv")
            nc.vector.tensor_scalar_max(deg_inv[:], deg_sb[:], 1.0)
            nc.vector.reciprocal(deg_inv[:], deg_inv[:])
            idx_sb = cpool.tile([128, S // 16], i16, tag="idx")
            nc.sync.dma_start(out=idx_sb[:], in_=idx_d.ap())
            xT_sb = cpool.tile([D_IN, SHARD], f32, tag="xT")
            nc.sync.dma_start(out=xT_sb[:], in_=xT_d.ap())

            # persistent per-layer strips (h^T), reused layer1 -> layer2
            hT = [ppool.tile([128, SHARD], bf16, tag=f"hT{k}", name=f"hT{k}")
                  for k in range(8)]

            tile_chunk0 = np.concatenate([[0], np.cumsum(KT)]).astype(int)

            def xw_phase(layer, lhsT_slices, rhs_tile, nm, Tloc):
                """xw_ext = feat @ W_ext -> Tloc; update node max nm [128,8]."""
                nc.sync.dma_start(out=Tloc[SHARD:SHARD1, :], in_=padrow[:])
                with (
                    tc.tile_pool(name=f"xwp{layer}", bufs=2) as wp,
                    tc.tile_pool(name=f"xwps{layer}", bufs=2, space="PSUM") as pp,
                ):
                    nch = [(0, 512), (512, 512), (1024, 64)]
                    for m in range(NT):
                        xw_sb = wp.tile([128, FCOLS], f32, tag="xw_sb")
                        for (n0, nn) in nch:
                            ps = pp.tile([128, 512], f32, tag="ps")
                            nks = len(lhsT_slices(m))
                            for ki, lt in enumerate(lhsT_slices(m)):
                                nc.tensor.matmul(ps[:, :nn], lt,
                                                 rhs_tile(ki)[:, n0:n0 + nn],
                                                 start=(ki == 0), stop=(ki == nks - 1))
                            nc.vector.tensor_copy(xw_sb[:, n0:n0 + nn], ps[:, :nn])
                        nc.vector.tensor_tensor(
                            out=nm[:], in0=nm[:], in1=xw_sb[:, HC:HC + 2 * H],
                            op=mybir.AluOpType.max)
                        nc.sync.dma_start(out=Tloc[m * 128:(m + 1) * 128, :],
                                          in_=xw_sb[:])

            def gat_layer(layer, stop_after=99):
                L = layer  # 0 or 1
                Tloc = dpool.tile([SHARD1, FCOLS], f32, tag="Tloc", name="Tloc")
                Tfull = dpool.tile([TROWS, FCOLS], f32, tag="Tfull", name="Tfull",
                                   addr_space="Shared")
                mx_in = dpool.tile([1, 12], f32, tag="mx_in", name="mx_in")
                mx_out = dpool.tile([1, 12], f32, tag="mx_out", name="mx_out",
                                    addr_space="Shared")
                nm = ppool.tile([128, 2 * H], f32, tag="nm")
                nc.vector.memset(nm[:], NEG)

                if L == 0:
                    W1e_sb = ppool.tile([D_IN, FCOLS], f32, tag="W1e")
                    nc.sync.dma_start(out=W1e_sb[:], in_=W1e_d.ap())
                    xw_phase(L, lambda m: [xT_sb[:, m * 128:(m + 1) * 128]],
                             lambda ki: W1e_sb, nm, Tloc)
                else:
                    with tc.tile_pool(name="w2p", bufs=1) as w2p:
                        W2e_sb = w2p.tile([128, 8, FCOLS], bf16, tag="W2e")
                        nc.sync.dma_start(
                            out=W2e_sb[:],
                            in_=W2e_d.ap().rearrange("(ko p) n -> p ko n", p=128))
                        xw_phase(L,
                                 lambda m: [hT[k][:, m * 128:(m + 1) * 128]
                                            for k in range(8)],
                                 lambda ki: W2e_sb[:, ki, :], nm, Tloc)

                # AllGather table (pad rows ride inside each shard)
                nc.gpsimd.collective_compute(
                    "AllGather", mybir.AluOpType.bypass, replica_groups=rg,
                    ins=[Tloc[:]], outs=[Tfull[:]])

                if stop_after <= 1:
                    return
                # ---- pre-pass: eaM for every chunk, per-tile sums, maxes
                with (
                    tc.tile_pool(name="prep", bufs=3) as ppre,
                    tc.tile_pool(name="prepp", bufs=2, space="PSUM") as ppsum,
                ):
                    eaM_all = ppool.tile([128, H, Ctot], f32, tag="eaM_all")
                    aeloop = ppool.tile([128, H, NT], f32, tag="aeloop")
                    ea_g = None
                    for c in range(Ctot):
                        g = c % G
                        if g == 0:
                            ea_g = ppre.tile([128, G, E_DIM], f32, tag="ea_g")
                            nc.sync.dma_start(
                                out=ea_g[:],
                                in_=ea_d.ap()[c * 128:(c + G) * 128, :]
                                .rearrange("(g p) d -> p g d", p=128))
                        tmp = ppre.tile([128, H, E_DIM], f32, tag="eamtmp")
                        nc.vector.tensor_tensor(
                            out=tmp[:],
                            in0=ea_g[:, g:g + 1, :].to_broadcast([128, H, E_DIM]),
                            in1=Mbc[:, L, :, :], op=mybir.AluOpType.mult)
                        nc.vector.reduce_sum(
                            out=eaM_all[:, :, c:c + 1], in_=tmp[:],
                            axis=mybir.AxisListType.X)
                    for t in range(NT):
                        c0, c1 = int(tile_chunk0[t]), int(tile_chunk0[t + 1])
                        nc.vector.reduce_sum(
                            out=aeloop[:, :, t:t + 1],
                            in_=eaM_all[:, :, c0:c1], axis=mybir.AxisListType.X)
                        nc.vector.tensor_scalar_mul(
                            aeloop[:, :, t:t + 1], aeloop[:, :, t:t + 1],
                            deg_inv[:, t:t + 1])
                    # maxes -> m_neg_col
                    mx_sb = ppre.tile([128, 12], f32, tag="mx_sb")
                    nc.vector.tensor_copy(mx_sb[:, 0:8], nm[:])
                    nc.vector.reduce_max(out=mx_sb[:, 8:12], in_=eaM_all[:],
                                         axis=mybir.AxisListType.X)
                    trp = ppsum.tile([12, 128], f32, tag="trp")
                    nc.tensor.transpose(trp[:], mx_sb[:], ident[:])
                    mx12 = ppre.tile([12, 128], f32, tag="mx12")
                    nc.vector.tensor_copy(mx12[:], trp[:])
                    mxc = ppre.tile([12, 1], f32, tag="mxc")
                    nc.vector.reduce_max(out=mxc[:], in_=mx12[:],
                                         axis=mybir.AxisListType.X)
                    nc.sync.dma_start(out=mx_in[:].rearrange("a b -> b a"),
                                      in_=mxc[:])
                    nc.gpsimd.collective_compute(
                        "AllReduce", mybir.AluOpType.max, replica_groups=rg,
                        ins=[mx_in[:]], outs=[mx_out[:]])
                    mrow = ppre.tile([1, 12], f32, tag="mrow")
                    nc.sync.dma_start(out=mrow[:], in_=mx_out[:])
                    t4 = ppre.tile([1, 4], f32, tag="t4")
                    nc.vector.tensor_add(t4[:], mrow[:, 0:4], mrow[:, 4:8])
                    nc.vector.tensor_add(t4[:], t4[:], mrow[:, 8:12])
                    nc.vector.tensor_scalar_max(t4[:], t4[:], 0.0)
                    mneg = ppre.tile([1, 1], f32, tag="mneg")
                    nc.vector.reduce_max(out=mneg[:], in_=t4[:],
                                         axis=mybir.AxisListType.X)
                    nc.vector.tensor_scalar_mul(mneg[:], mneg[:], -1.0)
                    bps = ppsum.tile([128, 1], f32, tag="bps")
                    nc.tensor.matmul(bps[:], ones_row[:], mneg[:],
                                     start=True, stop=True)
                    mnegc = ppool.tile([128, 1], f32, tag="mnegc")
                    nc.vector.tensor_copy(mnegc[:], bps[:])

                if stop_after <= 2:
                    return
                # ---- main pass
                with (
                    tc.tile_pool(name="gat", bufs=2) as gp,
                    tc.tile_pool(name="gat_s", bufs=3) as sp,
                    tc.tile_pool(name="hpool", bufs=2) as hp,
                    tc.tile_pool(name="gatps", bufs=2, space="PSUM") as agp,
                    tc.tile_pool(name="trps", bufs=2, space="PSUM") as trp2,
                ):
                    g_sb = None
                    for t in range(NT):
                        c0, c1 = int(tile_chunk0[t]), int(tile_chunk0[t + 1])
                        kt = c1 - c0
                        psum_t = agp.tile([128, HC], f32, tag="agg")
                        exp_all = sp.tile([128, H, KTmax], f32, tag="exp_all")
                        al_dst_t = sp.tile([128, H], f32, tag="al_dst")
                        for k in range(kt):
                            c = c0 + k
                            g = c % G
                            if g == 0:
                                g_sb = gp.tile([128, G, FCOLS], f32, tag="g_sb")
                                nc.gpsimd.dma_gather(
                                    out_ap=g_sb[:], in_ap=Tfull[:],
                                    idxs_ap=idx_sb[:, c * 8:(c + G) * 8],
                                    num_idxs=G * 128, num_idxs_reg=G * 128,
                                    elem_size=FCOLS)
                            if k == 0:
                                nc.vector.tensor_copy(
                                    al_dst_t[:], g_sb[:, g, HC + H:HC + 2 * H])
                            l0 = sp.tile([128, H], f32, tag="l0")
                            nc.vector.tensor_add(l0[:], g_sb[:, g, HC:HC + H],
                                                 al_dst_t[:])
                            adde = (aeloop[:, :, t:t + 1] if k == 0
                                    else eaM_all[:, :, c:c + 1])
                            nc.vector.tensor_tensor(
                                out=l0[:], in0=l0[:],
                                in1=adde, op=mybir.AluOpType.add)
                            l2 = sp.tile([128, H], f32, tag="l2")
                            nc.vector.tensor_scalar_mul(l2[:], l0[:], 0.2)
                            nc.vector.tensor_tensor(out=l2[:], in0=l2[:], in1=l0[:],
                                                    op=mybir.AluOpType.max)
                            nc.scalar.activation(
                                out=exp_all[:, :, k:k + 1], in_=l2[:],
                                func=mybir.ActivationFunctionType.Exp,
                                bias=mnegc[:], scale=1.0)
                            scaled = sp.tile([128, H, C], bf16, tag="scaled")
                            nc.vector.tensor_tensor(
                                out=scaled[:],
                                in0=g_sb[:, g, 0:HC].rearrange("p (h c) -> p h c", h=H),
                                in1=exp_all[:, :, k:k + 1].to_broadcast([128, H, C]),
                                op=mybir.AluOpType.mult)
                            sc2 = scaled[:].rearrange("p h c -> p (h c)")
                            nc.tensor.matmul(psum_t[:, 0:512], identb[:],
                                             sc2[:, 0:512],
                                             start=(k == 0), stop=(k == kt - 1))
                            nc.tensor.matmul(psum_t[:, 512:1024], identb[:],
                                             sc2[:, 512:1024],
                                             start=(k == 0), stop=(k == kt - 1))
                        # epilogue
                        s_t = sp.tile([128, H], f32, tag="s_t")
                        nc.vector.reduce_sum(out=s_t[:], in_=exp_all[:, :, 0:kt],
                                             axis=mybir.AxisListType.X)
                        nc.vector.tensor_scalar_add(s_t[:], s_t[:], 1e-16)
                        rec = sp.tile([128, H], f32, tag="rec")
                        nc.vector.reciprocal(rec[:], s_t[:])
                        h_sb = hp.tile([128, HC], f32, tag="h_sb")
                        for h in range(H):
                            nc.vector.tensor_scalar_mul(
                                h_sb[:, h * C:(h + 1) * C],
                                psum_t[:, h * C:(h + 1) * C], rec[:, h:h + 1])
                        nc.vector.tensor_tensor(out=h_sb[:], in0=h_sb[:],
                                                in1=bbc[:, L, :],
                                                op=mybir.AluOpType.add)
                        u_t = sp.tile([128, HC], f32, tag="u_t")
                        nc.vector.tensor_scalar_min(u_t[:], h_sb[:], 0.0)
                        nc.scalar.activation(
                            out=u_t[:], in_=u_t[:],
                            func=mybir.ActivationFunctionType.Exp)
                        nc.vector.tensor_scalar_max(h_sb[:], h_sb[:], 0.0)
                        nc.vector.tensor_add(h_sb[:], h_sb[:], u_t[:])
                        nc.vector.tensor_scalar_add(h_sb[:], h_sb[:], -1.0)
                        # transpose into hT strips
                        for k8 in range(8):
                            tp = trp2.tile([128, 128], f32, tag="tp")
                            nc.tensor.transpose(
                                tp[:], h_sb[:, k8 * 128:(k8 + 1) * 128], ident[:])
                            nc.vector.tensor_copy(
                                hT[k8][:, t * 128:(t + 1) * 128], tp[:])

            final_copy = []  # filled in final phase for mode 10
            gat_layer(0, stop_after)
            if stop_after >= 4:
                gat_layer(1, 99)
            do_final = stop_after >= 5

            # -------- final row-sharded Linear: out = h2 @ Wf + bf
            # (out DMAs are batched per n-chunk into [1280, NCH] strips; many
            #  small strided DMAs spread across the whole IO tensor faulted
            #  the device, strip-batched writes are fine)
            with (
                tc.tile_pool(name="fin", bufs=2) as fp,
                tc.tile_pool(name="finps", bufs=3, space="PSUM") as fpp,
            ):
                for n in range(NNCH if do_final else 0):
                    n0 = n * NCH
                    wf_sb = fp.tile([128, 8, NCH], bf16, tag="wf_sb")
                    nc.sync.dma_start(
                        out=wf_sb[:],
                        in_=Wf_d.ap()[:, n0:n0 + NCH]
                        .rearrange("(ko p) n -> p ko n", p=128))
                    bf_sb = fp.tile([128, NCH], f32, tag="bf_sb")
                    nc.sync.dma_start(out=bf_sb[:], in_=bf_d.ap()[:, n0:n0 + NCH])
                    strip = fp.tile([128, NT, NCH], f32, tag="strip")
                    for m in range(NT):
                        ps = fpp.tile([128, NCH], f32, tag="fin")
                        for k in range(8):
                            nc.tensor.matmul(
                                ps[:], hT[k][:, m * 128:(m + 1) * 128],
                                wf_sb[:, k, :], start=(k == 0), stop=(k == 7))
                        nc.vector.tensor_add(strip[:, m, :], ps[:], bf_sb[:])
                    nc.sync.dma_start(
                        out=out_d.ap()[:, n0:n0 + NCH]
                        .rearrange("(m p) n -> p m n", p=128),
                        in_=strip[:])

    nc.compile()
    return nc


# ------------------------------------------------------------------- driver

def kernel(**inputs):
    from concourse.bass_utils import run_bass_kernel_spmd

    x = np.asarray(inputs["x"], np.float32)
    ei = np.asarray(inputs["edge_index"])
    ef = np.asarray(inputs["edge_features"], np.float32)

    shards, KT, S, perm, inv = _build_shards(ei, ef)
    key = (S, tuple(int(k) for k in KT))
    if key not in _CACHE:
        _CACHE[key] = _build(KT, S)
    nc = _CACHE[key]

    W1e = _fold_weights(np.asarray(inputs["W1"], np.float32),
                        np.asarray(inputs["att_src1"], np.float32),
                        np.asarray(inputs["att_dst1"], np.float32))
    W2e = _fold_weights(np.asarray(inputs["W2"], np.float32),
                        np.asarray(inputs["att_src2"], np.float32),
                        np.asarray(inputs["att_dst2"], np.float32)
                        ).astype(ml_dtypes.bfloat16)
    M1 = _fold_edge(np.asarray(inputs["We1"], np.float32),
                    np.asarray(inputs["att_edge1"], np.float32))
    M2 = _fold_edge(np.asarray(inputs["We2"], np.float32),
                    np.asarray(inputs["att_edge2"], np.float32))
    Mbc = np.broadcast_to(
        np.stack([M1.T, M2.T])[None], (128, 2, H, E_DIM)).copy()  # [128,2,H,16]
    bbc = np.broadcast_to(
        np.stack([np.asarray(inputs["b1"], np.float32),
                  np.asarray(inputs["b2"], np.float32)])[None],
        (128, 2, HC)).copy()
    Wf = np.ascontiguousarray(
        np.asarray(inputs["Wf"], np.float32).astype(ml_dtypes.bfloat16))
    bfbc = np.broadcast_to(np.asarray(inputs["bf"], np.float32)[None],
                           (128, FOUT)).copy()

    xpad = np.zeros((NPAD, D_IN), np.float32)
    xpad[:N] = x
    x_new = xpad[np.where(perm < N, perm, 0)]
    x_new[perm >= N] = 0.0

    in_maps = []
    for c in range(NCORES):
        src_slots, ea_slots, deg = shards[c]
        idx16 = np.tile(src_slots.astype(np.int16).reshape(S // 16, 16).T, (8, 1)).copy()
        in_maps.append({
            "x_T": np.ascontiguousarray(x_new[c * SHARD:(c + 1) * SHARD].T),
            "W1e": W1e, "W2e": W2e, "M_bc": Mbc, "b_bc": bbc,
            "deg": np.ascontiguousarray(deg.reshape(NT, 128).T),
            "idx16": idx16, "ea_sl": ea_slots,
            "Wf": Wf, "bf_bc": bfbc,
        })

    trace = os.environ.get("KERNEL_TRACE", "") == "1"
    res = run_bass_kernel_spmd(nc, in_maps, core_ids=list(range(NCORES)),
                               trace=trace,
                               trace_cores=[0] if trace else None)
    global _last_results
    _last_results = res
    out_new = np.concatenate([res.results[c]["out"] for c in range(NCORES)],
                             axis=0)          # [NPAD, FOUT] in new node order
    return out_new[inv[:N]]


_last_results = None



# revision 6
# speedup vs baseline: 1.4133x; 1.4133x over previous
"""Trainium2 Bass kernel for nn_GATSTEMEncoder (2-layer GAT + Linear 1024->25088).

Self-contained: hardcodes all shapes; builds + compiles the Bass program on
first call (cached per graph structure) and runs it SPMD on 8 NeuronCores.

Design (v2):
- Nodes relabeled so core c owns new ids [c*1280,(c+1)*1280), degree-sorted
  within core. Edges live with their dst core as a slot-CSR (slot 0 = self
  loop). Per-edge gathers use dma_gather on a bf16 feature table
  T [10248, 1032] = [xw(1024) | al_src(4) | al_dst(4)]; row c*1281+1280 is a
  pad row with al_src = -1e9 so padded slots contribute exp()==0 exactly.
- Layer 1: every core computes the FULL xw1 table locally (cheap bf16 matmul)
  -> no AllGather and no AllReduce for layer 1.
- Layer 2: sharded xw2 + one bf16 AllGather + tiny AllReduce for the softmax
  stabilization bound.
- Per-edge attention logits from edge_features are HOST-precomputed (they only
  depend on inputs): alE[slot,h], with the self-loop slot holding the
  fill_value='mean' logit. No on-device edge-feature processing at all.
- Final Linear row-sharded: each core computes its 1280 rows x 25088 cols.
"""
import os
import sys
import numpy as np
import ml_dtypes

for p in ("/opt/trn_rl_repo", "/root/.axon_site", "/root/.axon_site/_ro/trn_rl_repo"):
    if p not in sys.path:
        sys.path.append(p)

H, C = 4, 256
HC = H * C
N = 10000
NPAD = 10240
NCORES = 8
SHARD = NPAD // NCORES          # 1280
NT = SHARD // 128               # 10 tiles/core
SHARD1 = 1281                   # shard rows incl. inline pad row
TROWS = NCORES * SHARD1         # 10248 table rows
PAD_ROW = 1280                  # pad row of shard 0 (any shard works)
NEG = -1.0e9
D_IN = 128
E_DIM = 16
FCOLS = 1152                    # xw(1024) | al_src(4) | al_dst(4) | pad, bf16
                                # (gather elem bytes must be %256: 1152*2=2304)
NBLK = NPAD // 128              # 80 blocks for the full xw1 table
FOUT = 25088
G = 8                           # slot-chunks per dma_gather call
NCH = 512                       # final matmul N-chunk (25088 = 49*512)
NNCH = FOUT // NCH


# ----------------------------------------------------------------- host prep

def _fold_weights(W, a_src, a_dst):
    din = W.shape[0]
    Wr = W.reshape(din, H, C)
    W_ext = np.zeros((din, FCOLS), np.float32)
    W_ext[:, :HC] = W
    W_ext[:, HC:HC + H] = np.einsum('dhc,hc->dh', Wr, a_src)
    W_ext[:, HC + H:HC + 2 * H] = np.einsum('dhc,hc->dh', Wr, a_dst)
    return W_ext


def _fold_edge(We, a_edge):
    return np.einsum('dhc,hc->dh', We.reshape(E_DIM, H, C), a_edge).astype(np.float32)


def _build_shards(edge_index, edge_features, M1, M2):
    """Slot-CSR per dst core + host-precomputed per-slot edge-attn logits."""
    src = np.asarray(edge_index[0], np.int64)
    dst = np.asarray(edge_index[1], np.int64)
    E = src.shape[0]

    order = np.argsort(dst, kind='stable')
    src_s = src[order]
    counts = np.bincount(dst[order], minlength=N)
    starts = np.concatenate([[0], np.cumsum(counts)])
    counts_pad = np.concatenate([counts, np.zeros(NPAD - N, np.int64)])

    perm = np.empty(NPAD, np.int64)
    for c in range(NCORES):
        lo = c * SHARD
        d = counts_pad[lo:lo + SHARD]
        perm[lo:lo + SHARD] = lo + np.argsort(-d, kind='stable')
    inv = np.empty(NPAD, np.int64)
    inv[perm] = np.arange(NPAD)
    deg_new = counts_pad[perm]

    KT = np.zeros(NT, np.int64)
    for t in range(NT):
        mx = 0
        for c in range(NCORES):
            d = deg_new[c * SHARD + t * 128: c * SHARD + (t + 1) * 128]
            mx = max(mx, int(d.max()))
        KT[t] = mx + 1
    KT[-1] += (-int(KT.sum())) % G
    S = int(KT.sum()) * 128

    # per-edge and per-node (loop) attention logits, original order
    alE_e = [edge_features @ M1, edge_features @ M2]        # [E,H] each
    loop_al = []
    for l in range(2):
        acc = np.zeros((N, H), np.float32)
        np.add.at(acc, dst, alE_e[l])
        loop_al.append(acc / np.maximum(counts, 1.0)[:, None])
    aeMax = np.array([max(alE_e[l].max(), loop_al[l].max(), 0.0) * np.ones(H)
                      for l in range(2)], np.float32)  # conservative per-head
    # per-head exact maxes
    aeMax = np.stack([
        np.maximum(np.maximum(alE_e[0].max(0), loop_al[0].max(0)), 0.0),
        np.maximum(np.maximum(alE_e[1].max(0), loop_al[1].max(0)), 0.0)],
        axis=0).astype(np.float32)                     # [2,H]

    def remap(i):
        return (i // SHARD) * SHARD1 + (i % SHARD)

    shards = []
    for c in range(NCORES):
        src_slots = np.full(S, PAD_ROW, np.int64)
        alE_slots = np.zeros((2, S, H), np.float32)
        base = 0
        for t in range(NT):
            kt = int(KT[t])
            for p in range(128):
                nid_new = c * SHARD + t * 128 + p
                nid_old = perm[nid_new]
                if nid_old >= N:
                    continue
                src_slots[base + p] = remap(nid_new)
                alE_slots[0, base + p] = loop_al[0][nid_old]
                alE_slots[1, base + p] = loop_al[1][nid_old]
                d = int(counts_pad[nid_old])
                if d > 0:
                    e0 = starts[nid_old]
                    idxs = base + (np.arange(d) + 1) * 128 + p
                    src_slots[idxs] = remap(inv[src_s[e0:e0 + d]])
                    alE_slots[0, idxs] = alE_e[0][order[e0:e0 + d]]
                    alE_slots[1, idxs] = alE_e[1][order[e0:e0 + d]]
            base += kt * 128
        # device layout [128, Ctot, H]
        Ctot = S // 128
        alE_dev = alE_slots.reshape(2, Ctot, 128, H).transpose(0, 2, 1, 3).copy()
        shards.append((src_slots.astype(np.int32), alE_dev))
    return shards, KT, S, perm, inv, aeMax


# --------------------------------------------------------------- bass build

_CACHE = {}


def _build(KT, S):
    import concourse.bass as bass
    import concourse.mybir as mybir
    import concourse.tile as tile
    from concourse import bacc
    from concourse.masks import make_identity

    f32 = mybir.dt.float32
    bf16 = mybir.dt.bfloat16
    i16 = mybir.dt.int16
    Ctot = S // 128
    NGRP = Ctot // G
    KTmax = int(max(KT))
    chunk0 = np.concatenate([[0], np.cumsum(KT)]).astype(int)
    rg = [list(range(NCORES))]
    AF = mybir.ActivationFunctionType
    OP = mybir.AluOpType

    nc = bacc.Bacc("TRN2", target_bir_lowering=False, debug=False,
                   num_devices=NCORES)

    # -------- I/O
    xT_d = nc.dram_tensor("x_T", [D_IN, NPAD], bf16, kind="ExternalInput")
    W1e_d = nc.dram_tensor("W1e", [D_IN, FCOLS], bf16, kind="ExternalInput")
    W2e_d = nc.dram_tensor("W2e", [HC, FCOLS], bf16, kind="ExternalInput")
    alE1_d = nc.dram_tensor("alE1", [128, Ctot, H], f32, kind="ExternalInput")
    alE2_d = nc.dram_tensor("alE2", [128, Ctot, H], f32, kind="ExternalInput")
    mc_d = nc.dram_tensor("mconst", [1, 16], f32, kind="ExternalInput")
    bbc_d = nc.dram_tensor("b_bc", [128, 2, HC], f32, kind="ExternalInput")
    idx_d = nc.dram_tensor("idx16", [128, S // 16], i16, kind="ExternalInput")
    Wf_d = nc.dram_tensor("Wf", [HC, FOUT], bf16, kind="ExternalInput")
    bf_d = nc.dram_tensor("bf_bc", [128, FOUT], f32, kind="ExternalInput")
    out_d = nc.dram_tensor("out", [SHARD, FOUT], f32, kind="ExternalOutput")

    with tile.TileContext(nc) as tc:
        with (
            tc.tile_pool(name="const", bufs=1) as cpool,
            tc.tile_pool(name="dram", bufs=1, space="DRAM") as dpool,
            tc.tile_pool(name="persist", bufs=1) as ppool,
        ):
            # ---- constants
            ident = cpool.tile([128, 128], f32, tag="ident")
            make_identity(nc, ident[:])
            identb = cpool.tile([128, 128], bf16, tag="identb")
            make_identity(nc, identb[:])
            ones_row = cpool.tile([1, 128], f32, tag="ones_row")
            nc.vector.memset(ones_row[:], 1.0)
            padrow = cpool.tile([1, FCOLS], bf16, tag="padrow")
            nc.vector.memset(padrow[:], 0.0)
            nc.vector.memset(padrow[:, HC:HC + H], NEG)
            mc_sb = cpool.tile([1, 16], f32, tag="mc")
            nc.sync.dma_start(out=mc_sb[:], in_=mc_d.ap())
            bbc = cpool.tile([128, 2, HC], f32, tag="bbc")
            nc.sync.dma_start(out=bbc[:], in_=bbc_d.ap())
            idx_sb = cpool.tile([128, S // 16], i16, tag="idx")
            nc.sync.dma_start(out=idx_sb[:], in_=idx_d.ap())
            alE_sb = [cpool.tile([128, Ctot, H], f32, tag=f"alE{l}", name=f"alE{l}")
                      for l in range(2)]
            nc.sync.dma_start(out=alE_sb[0][:], in_=alE1_d.ap())
            nc.sync.dma_start(out=alE_sb[1][:], in_=alE2_d.ap())
            xT_sb = cpool.tile([D_IN, NPAD], bf16, tag="xT")
            nc.sync.dma_start(out=xT_sb[:], in_=xT_d.ap())
            W1e_sb = cpool.tile([D_IN, FCOLS], bf16, tag="W1e")
            nc.sync.dma_start(out=W1e_sb[:], in_=W1e_d.ap())
            W2e_sb = cpool.tile([128, 8, FCOLS], bf16, tag="W2e")
            nc.sync.dma_start(
                out=W2e_sb[:],
                in_=W2e_d.ap().rearrange("(ko p) n -> p ko n", p=128))

            # persistent strips (h^T), reused layer1 -> layer2
            hT = [ppool.tile([128, SHARD], bf16, tag=f"hT{k}", name=f"hT{k}")
                  for k in range(8)]
            nm = [ppool.tile([128, 2 * H], f32, tag=f"nm{l}", name=f"nm{l}") for l in range(2)]
            mnegc = [ppool.tile([128, 1], f32, tag=f"mnegc{l}", name=f"mnegc{l}") for l in range(2)]

            # DRAM tables
            T1 = dpool.tile([TROWS, FCOLS], bf16, tag="T1", name="T1")
            Tloc2 = dpool.tile([SHARD1, FCOLS], bf16, tag="Tloc2", name="Tloc2")
            Tfull2 = dpool.tile([TROWS, FCOLS], bf16, tag="Tfull2", name="Tfull2",
                                addr_space="Shared")
            mx_in = dpool.tile([1, 8], f32, tag="mx_in", name="mx_in")
            mx_out = dpool.tile([1, 8], f32, tag="mx_out", name="mx_out",
                                addr_space="Shared")

            nc.vector.memset(nm[0][:], NEG)
            nc.vector.memset(nm[1][:], NEG)

            # ---------------- helper: exp-bound -> mnegc[L]
            def make_bound(L, allreduce, mpool, mpsum):
                trp = mpsum.tile([8, 128], f32, tag="btr")
                nc.tensor.transpose(trp[:], nm[L][:], ident[:])
                mx8 = mpool.tile([8, 128], f32, tag="mx8")
                nc.vector.tensor_copy(mx8[:], trp[:])
                mxc = mpool.tile([8, 1], f32, tag="mxc")
                nc.vector.reduce_max(out=mxc[:], in_=mx8[:],
                                     axis=mybir.AxisListType.X)
                nc.sync.dma_start(out=mx_in[:].rearrange("a b -> b a"),
                                  in_=mxc[:])
                if allreduce:
                    nc.gpsimd.collective_compute(
                        "AllReduce", OP.max, replica_groups=rg,
                        ins=[mx_in[:]], outs=[mx_out[:]])
                    src_t = mx_out
                else:
                    src_t = mx_in
                mrow = mpool.tile([1, 8], f32, tag="mrow")
                nc.sync.dma_start(out=mrow[:], in_=src_t[:])
                t4 = mpool.tile([1, 4], f32, tag="t4")
                nc.vector.tensor_add(t4[:], mrow[:, 0:4], mrow[:, 4:8])
                nc.vector.tensor_tensor(out=t4[:], in0=t4[:],
                                        in1=mc_sb[:, L * 4:L * 4 + 4], op=OP.add)
                nc.vector.tensor_scalar_max(t4[:], t4[:], 0.0)
                mneg = mpool.tile([1, 1], f32, tag="mneg")
                nc.vector.reduce_max(out=mneg[:], in_=t4[:],
                                     axis=mybir.AxisListType.X)
                nc.vector.tensor_scalar_mul(mneg[:], mneg[:], -1.0)
                bps = mpsum.tile([128, 1], f32, tag="bps")
                nc.tensor.matmul(bps[:], ones_row[:], mneg[:],
                                 start=True, stop=True)
                nc.vector.tensor_copy(mnegc[L][:], bps[:])

            # ---------------- phase X1: full xw1 table on every core
            with (
                tc.tile_pool(name="x1w", bufs=2) as wp,
                tc.tile_pool(name="x1p", bufs=2, space="PSUM") as pp,
            ):
                for m in range(NBLK):
                    lt = xT_sb[:, m * 128:(m + 1) * 128]
                    ps0 = pp.tile([128, 512], f32, tag="ps0")
                    nc.tensor.matmul(ps0[:], lt, W1e_sb[:, 0:512],
                                     start=True, stop=True)
                    ps1 = pp.tile([128, 512], f32, tag="ps1")
                    nc.tensor.matmul(ps1[:], lt, W1e_sb[:, 512:1024],
                                     start=True, stop=True)
                    ps2 = pp.tile([128, 8], f32, tag="ps2")
                    nc.tensor.matmul(ps2[:], lt, W1e_sb[:, 1024:1032],
                                     start=True, stop=True)
                    sb = wp.tile([128, FCOLS], bf16, tag="xsb")
                    nc.vector.tensor_copy(sb[:, 0:512], ps0[:])
                    nc.scalar.activation(out=sb[:, 512:1024], in_=ps1[:],
                                         func=AF.Copy)
                    nc.vector.tensor_copy(sb[:, 1024:1032], ps2[:])
                    nc.vector.tensor_tensor(out=nm[0][:], in0=nm[0][:],
                                            in1=ps2[:], op=OP.max)
                    r0 = (m // NT) * SHARD1 + (m % NT) * 128
                    nc.sync.dma_start(out=T1[r0:r0 + 128, :], in_=sb[:])
                for c in range(NCORES):
                    nc.sync.dma_start(
                        out=T1[c * SHARD1 + SHARD:c * SHARD1 + SHARD1, :],
                        in_=padrow[:])
            with (
                tc.tile_pool(name="b1", bufs=1) as b1p,
                tc.tile_pool(name="b1ps", bufs=1, space="PSUM") as b1ps,
            ):
                make_bound(0, False, b1p, b1ps)

            # ---------------- edge loop (shared for both layers)
            def edge_layer(L, table, on_tile_done):
                with (
                    tc.tile_pool(name=f"gp{L}", bufs=2) as gp,
                    tc.tile_pool(name=f"mp{L}", bufs=2) as mp,
                    tc.tile_pool(name=f"sp{L}", bufs=3) as sp,
                    tc.tile_pool(name=f"st{L}", bufs=2) as stp,
                    tc.tile_pool(name=f"ep{L}", bufs=2) as ep,
                    tc.tile_pool(name=f"ag{L}", bufs=2, space="PSUM") as agp,
                    tc.tile_pool(name=f"tr{L}", bufs=2, space="PSUM") as trp,
                ):
                    state = {}
                    for grp in range(NGRP):
                        cb0 = grp * G
                        g_sb = gp.tile([128, G, FCOLS], bf16, tag="g_sb")
                        nc.gpsimd.dma_gather(
                            out_ap=g_sb[:], in_ap=table[:],
                            idxs_ap=idx_sb[:, cb0 * 8:(cb0 + G) * 8],
                            num_idxs=G * 128, num_idxs_reg=G * 128,
                            elem_size=FCOLS)
                        exp_bf = sp.tile([128, G, H], bf16, tag="exp_bf")
                        segs = []
                        for t in range(NT):
                            a = max(cb0, int(chunk0[t]))
                            b = min(cb0 + G, int(chunk0[t + 1]))
                            if a < b:
                                segs.append((t, a, b))
                        for (t, ca, cb) in segs:
                            c0, c1 = int(chunk0[t]), int(chunk0[t + 1])
                            if ca == c0:
                                st = {
                                    'psum': agp.tile([128, HC], f32, tag="agg", name="agg"),
                                    'expt': stp.tile([128, H, KTmax], f32,
                                                     tag="expt", name="expt"),
                                    'ald': stp.tile([128, 1, H], f32,
                                                    tag="ald", name="ald"),
                                }
                                state[t] = st
                                nc.vector.tensor_copy(
                                    st['ald'][:, 0, :],
                                    g_sb[:, ca - cb0, HC + H:HC + 2 * H])
                            st = state[t]
                            n = cb - ca
                            ga = ca - cb0
                            l0 = sp.tile([128, G, H], f32, tag="l0")
                            nc.vector.tensor_tensor(
                                out=l0[:, 0:n], in0=g_sb[:, ga:ga + n, HC:HC + H],
                                in1=alE_sb[L][:, ca:cb, :], op=OP.add)
                            nc.vector.tensor_tensor(
                                out=l0[:, 0:n], in0=l0[:, 0:n],
                                in1=st['ald'][:].to_broadcast([128, n, H]),
                                op=OP.add)
                            nc.vector.scalar_tensor_tensor(
                                out=l0[:, 0:n], in0=l0[:, 0:n], scalar=0.2,
                                in1=l0[:, 0:n], op0=OP.mult, op1=OP.max)
                            k0 = ca - c0
                            nc.scalar.activation(
                                out=st['expt'][:, :, k0:k0 + n],
                                in_=l0[:, 0:n].rearrange("p g h -> p h g"),
                                func=AF.Exp, bias=mnegc[L][:], scale=1.0)
                            nc.scalar.activation(
                                out=exp_bf[:, ga:ga + n, :],
                                in_=l0[:, 0:n],
                                func=AF.Exp, bias=mnegc[L][:], scale=1.0)
                        scaled = mp.tile([128, G, H, C], bf16, tag="scaled")
                        nc.vector.tensor_tensor(
                            out=scaled[:],
                            in0=g_sb[:, :, 0:HC].rearrange("p g (h c) -> p g h c", h=H),
                            in1=exp_bf[:].to_broadcast([128, G, H, C]),
                            op=OP.mult)
                        sc2 = scaled[:].rearrange("p g h c -> p (g h c)")
                        for (t, ca, cb) in segs:
                            st = state[t]
                            c0, c1 = int(chunk0[t]), int(chunk0[t + 1])
                            for c in range(ca, cb):
                                g = c - cb0
                                nc.tensor.matmul(
                                    st['psum'][:, 0:512], identb[:],
                                    sc2[:, g * HC:g * HC + 512],
                                    start=(c == c0), stop=(c == c1 - 1))
                                nc.tensor.matmul(
                                    st['psum'][:, 512:1024], identb[:],
                                    sc2[:, g * HC + 512:(g + 1) * HC],
                                    start=(c == c0), stop=(c == c1 - 1))
                            if cb == c1:
                                # ---- tile epilogue
                                kt = c1 - c0
                                s_t = ep.tile([128, H], f32, tag="s_t")
                                nc.vector.reduce_sum(
                                    out=s_t[:], in_=st['expt'][:, :, 0:kt],
                                    axis=mybir.AxisListType.X)
                                nc.vector.tensor_scalar_add(s_t[:], s_t[:], 1e-16)
                                rec = ep.tile([128, H], f32, tag="rec")
                                nc.vector.reciprocal(rec[:], s_t[:])
                                h_sb = ep.tile([128, HC], f32, tag="h_sb")
                                for h in range(H):
                                    nc.scalar.activation(
                                        out=h_sb[:, h * C:(h + 1) * C],
                                        in_=st['psum'][:, h * C:(h + 1) * C],
                                        func=AF.Copy, scale=rec[:, h:h + 1])
                                nc.vector.tensor_tensor(
                                    out=h_sb[:], in0=h_sb[:], in1=bbc[:, L, :],
                                    op=OP.add)
                                u = ep.tile([128, HC], f32, tag="u")
                                nc.vector.tensor_scalar_min(u[:], h_sb[:], 0.0)
                                nc.scalar.activation(out=u[:], in_=u[:],
                                                     func=AF.Exp)
                                h2 = ep.tile([128, HC], bf16, tag="h2")
                                nc.vector.scalar_tensor_tensor(
                                    out=h2[:], in0=u[:], scalar=-1.0,
                                    in1=h_sb[:], op0=OP.add, op1=OP.max)
                                for k8 in range(8):
                                    tp = trp.tile([128, 128], bf16, tag="tp")
                                    nc.tensor.transpose(
                                        tp[:], h2[:, k8 * 128:(k8 + 1) * 128],
                                        identb[:])
                                    nc.vector.tensor_copy(
                                        hT[k8][:, t * 128:(t + 1) * 128], tp[:])
                                del state[t]
                                if on_tile_done is not None:
                                    on_tile_done(t)

            # ---------------- layer 1 (with xw2 inlined per finished tile)
            with (
                tc.tile_pool(name="x2w", bufs=2) as w2p,
                tc.tile_pool(name="x2p", bufs=1, space="PSUM") as x2pp,
            ):
                def xw2_block(t):
                    sb = w2p.tile([128, FCOLS], bf16, tag="t2sb")
                    for (n0, nn, tg) in ((0, 512, "xq0"), (512, 512, "xq0"),
                                         (1024, 8, "xq2")):
                        ps = x2pp.tile([128, nn], f32, tag=tg, name=tg)
                        for k in range(8):
                            nc.tensor.matmul(
                                ps[:, 0:nn],
                                hT[k][:, t * 128:(t + 1) * 128],
                                W2e_sb[:, k, n0:n0 + nn],
                                start=(k == 0), stop=(k == 7))
                        if n0 == 1024:
                            nc.vector.tensor_tensor(out=nm[1][:], in0=nm[1][:],
                                                    in1=ps[:], op=OP.max)
                            nc.vector.tensor_copy(sb[:, n0:n0 + nn], ps[:])
                        elif n0 == 0:
                            nc.vector.tensor_copy(sb[:, n0:n0 + nn], ps[:])
                        else:
                            nc.scalar.activation(out=sb[:, n0:n0 + nn],
                                                 in_=ps[:], func=AF.Copy)
                    nc.sync.dma_start(out=Tloc2[t * 128:(t + 1) * 128, :],
                                      in_=sb[:])

                edge_layer(0, T1, xw2_block)
                nc.sync.dma_start(out=Tloc2[SHARD:SHARD1, :], in_=padrow[:])

            # bound for layer 2 (needs AllReduce) + AllGather of the table
            with (
                tc.tile_pool(name="b2", bufs=1) as b2p,
                tc.tile_pool(name="b2ps", bufs=1, space="PSUM") as b2ps,
            ):
                make_bound(1, True, b2p, b2ps)
            nc.gpsimd.collective_compute(
                "AllGather", OP.bypass, replica_groups=rg,
                ins=[Tloc2[:]], outs=[Tfull2[:]])

            # ---------------- layer 2
            edge_layer(1, Tfull2, None)

            # ---------------- final row-sharded Linear: out = h2 @ Wf + bf
            with (
                tc.tile_pool(name="fin", bufs=2) as fp,
                tc.tile_pool(name="finps", bufs=3, space="PSUM") as fpp,
            ):
                for n in range(NNCH):
                    n0 = n * NCH
                    wf_sb = fp.tile([128, 8, NCH], bf16, tag="wf_sb")
                    nc.sync.dma_start(
                        out=wf_sb[:],
                        in_=Wf_d.ap()[:, n0:n0 + NCH]
                        .rearrange("(ko p) n -> p ko n", p=128))
                    bf_sb = fp.tile([128, NCH], f32, tag="bf_sb")
                    nc.sync.dma_start(out=bf_sb[:], in_=bf_d.ap()[:, n0:n0 + NCH])
                    strip = fp.tile([128, NT, NCH], f32, tag="strip")
                    for m in range(NT):
                        ps = fpp.tile([128, NCH], f32, tag="fin")
                        for k in range(8):
                            nc.tensor.matmul(
                                ps[:], hT[k][:, m * 128:(m + 1) * 128],
                                wf_sb[:, k, :], start=(k == 0), stop=(k == 7))
                        nc.vector.tensor_add(strip[:, m, :], ps[:], bf_sb[:])
                    nc.sync.dma_start(
                        out=out_d.ap()[:, n0:n0 + NCH]
                        .rearrange("(m p) n -> p m n", p=128),
                        in_=strip[:])

    nc.compile()
    return nc


# ------------------------------------------------------------------- driver

def kernel(**inputs):
    from concourse.bass_utils import run_bass_kernel_spmd

    x = np.asarray(inputs["x"], np.float32)
    ei = np.asarray(inputs["edge_index"])
    ef = np.asarray(inputs["edge_features"], np.float32)

    M1 = _fold_edge(np.asarray(inputs["We1"], np.float32),
                    np.asarray(inputs["att_edge1"], np.float32))
    M2 = _fold_edge(np.asarray(inputs["We2"], np.float32),
                    np.asarray(inputs["att_edge2"], np.float32))
    shards, KT, S, perm, inv, aeMax = _build_shards(ei, ef, M1, M2)
    key = (S, tuple(int(k) for k in KT))
    if key not in _CACHE:
        _CACHE[key] = _build(KT, S)
    nc = _CACHE[key]

    W1e = _fold_weights(np.asarray(inputs["W1"], np.float32),
                        np.asarray(inputs["att_src1"], np.float32),
                        np.asarray(inputs["att_dst1"], np.float32)
                        ).astype(ml_dtypes.bfloat16)
    W2e = _fold_weights(np.asarray(inputs["W2"], np.float32),
                        np.asarray(inputs["att_src2"], np.float32),
                        np.asarray(inputs["att_dst2"], np.float32)
                        ).astype(ml_dtypes.bfloat16)
    mconst = np.zeros((1, 16), np.float32)
    mconst[0, 0:4] = aeMax[0]
    mconst[0, 4:8] = aeMax[1]
    bbc = np.broadcast_to(
        np.stack([np.asarray(inputs["b1"], np.float32),
                  np.asarray(inputs["b2"], np.float32)])[None],
        (128, 2, HC)).copy()
    Wf = np.ascontiguousarray(
        np.asarray(inputs["Wf"], np.float32).astype(ml_dtypes.bfloat16))
    bfbc = np.broadcast_to(np.asarray(inputs["bf"], np.float32)[None],
                           (128, FOUT)).copy()

    xpad = np.zeros((NPAD, D_IN), np.float32)
    xpad[:N] = x
    x_new = xpad[np.where(perm < N, perm, 0)]
    x_new[perm >= N] = 0.0
    xT = np.ascontiguousarray(x_new.T).astype(ml_dtypes.bfloat16)

    in_maps = []
    for c in range(NCORES):
        src_slots, alE_dev = shards[c]
        idx16 = np.tile(src_slots.astype(np.int16).reshape(S // 16, 16).T,
                        (8, 1)).copy()
        in_maps.append({
            "x_T": xT, "W1e": W1e, "W2e": W2e,
            "alE1": np.ascontiguousarray(alE_dev[0]),
            "alE2": np.ascontiguousarray(alE_dev[1]),
            "mconst": mconst, "b_bc": bbc,
            "idx16": idx16,
            "Wf": Wf, "bf_bc": bfbc,
        })

    trace = os.environ.get("KERNEL_TRACE", "") == "1"
    res = run_bass_kernel_spmd(nc, in_maps, core_ids=list(range(NCORES)),
                               trace=trace,
                               trace_cores=[0] if trace else None)
    global _last_results
    _last_results = res
    out_new = np.concatenate([res.results[c]["out"] for c in range(NCORES)],
                             axis=0)          # [NPAD, FOUT] in new node order
    return out_new[inv[:N]]


_last_results = None
